# revision 29
# baseline (speedup 1.0000x reference)
"""Trainium2 Bass kernel for nn_CVT_12446815223882 (sparse cylindrical attention).

Pipeline: unproject depth -> cylinder coords (theta, z) per camera face,
truncated-Gaussian similarity over (theta, z) with inv_cov = (1/0.002) I,
out = sim @ (x @ Wv.T).

Key structure exploited: with cov = 0.002 I and TRUNC = 1.2, the mask
dist2 <= TRUNC^2 keeps only pairs with |dtheta|, |dz| <= 0.0537 -- the
similarity matrix is ~99.9% sparse. Host sorts each face's 1600 points by
theta; surviving pairs then live in a +-36-position band. The device
computes a 256-wide circular band of sim per 128-point chunk and does
banded matmuls. Wrap pairs (theta ~ +-pi) are handled by extending the
sorted arrays with +-2pi-shifted copies -- no mod/wrap ops on device.

Banded layout: out-blocks q live on the 128 grid (rows [128q, 128q+128));
similarity tiles live on a 64-shifted grid (chunks c_p = [128p-64,
128p+64)). Window tile W[p] = sim(c_p rows as partitions, cols [128(p-1),
128(p+1)) as free) covers all |row-col| <= 64 pairs; out[q] contracts
over shifted chunks {q, q+1}, whose lhsT tiles are plain slices of W[q]
and W[q+1] by symmetry of sim.

Work split: each face = 13 out-blocks; 2 overlapping segments of 7 blocks
-> 24 segments = 8 NeuronCores x 3 segments. One SPMD program.
"""

import numpy as np
import ml_dtypes

import concourse.bass as bass
import concourse.bacc as bacc
import concourse.tile as tile
import concourse.mybir as mybir
from concourse.bass_utils import run_bass_kernel_spmd

# ---- problem constants (from the nn.Module definition) ----
R_FIXED = 2.0
TRUNC = 1.2
RADIUS = 1.0
DEPTH_THRESHOLD = 500.0
CENTER = np.array([0.0, 0.0, 1.5], dtype=np.float32)

B_, NC_, H_, W_ = 2, 6, 40, 40
HW = H_ * W_          # 1600
C = 256
NFACE = B_ * NC_      # 12
NBLK = 13             # out row blocks of 128 per face (last 64 real)
SEGJ = 7              # out blocks per segment
NCH = SEGJ + 1        # shifted sim/v chunks per segment
NSEG = NFACE * 2      # 24 (blocks 0..6 and 6..12; block 6 overlaps)
NCORES = 8
SEG_PER_CORE = NSEG // NCORES  # 3
WINW = 256            # window width per shifted chunk
SEGW = 128 * SEGJ         # 896: col span actually read by A/B windows
XTW = 128 * NCH           # 1024: x cols per segment (shifted grid)
BIGW = WINW * NCH         # 2048: fused elementwise width

LAST_RESULT = None
BF16 = np.float16


def _geometry(depth, K, T):
    """Replicate reference.py unprojection + cylinder projection bit-for-bit
    (jax on CPU, same op sequence) -> theta, z per face [NFACE, HW] f32."""
    import jax
    import jax.numpy as jnp

    cpu = jax.devices("cpu")[0]
    with jax.default_device(cpu):
        depth = jnp.asarray(np.asarray(depth), jnp.float32)
        K = jnp.asarray(np.asarray(K), jnp.float32)
        T = jnp.asarray(np.asarray(T), jnp.float32)
        Bb, NC, _, H, W = depth.shape
        ys, xs = jnp.meshgrid(jnp.arange(H, dtype=jnp.float32),
                              jnp.arange(W, dtype=jnp.float32), indexing="ij")
        d = depth[:, :, 0]
        fx = K[:, :, 0, 0][..., None, None]
        fy = K[:, :, 1, 1][..., None, None]
        cx = K[:, :, 0, 2][..., None, None]
        cy = K[:, :, 1, 2][..., None, None]
        X = (xs - cx) / fx * d
        Y = (ys - cy) / fy * d
        pts_cam = jnp.stack([X, Y, d], axis=-1).reshape(Bb, NC, H * W, 3)
        valid = ((d > 0) & (d <= DEPTH_THRESHOLD)).reshape(Bb, NC, H * W)
        pts_h = jnp.concatenate([pts_cam, jnp.ones_like(pts_cam[..., :1])], axis=-1)
        pts_ego = jnp.einsum("bnkj,bnij->bnki", pts_h, T)[..., :3]
        pts_ego = jnp.where(valid[..., None], pts_ego, 0.0)
        center = jnp.asarray(CENTER)
        v = pts_ego - center
        r = jnp.sqrt(v[..., 0] ** 2 + v[..., 1] ** 2)
        r_safe = jnp.where(r == 0, 1e-6, r)
        t = (R_FIXED / r_safe)[..., None]
        proj = center - t * v
        theta = jnp.arctan2(proj[..., 1] - center[1], proj[..., 0] - center[0])
        height = proj[..., 2] - center[2]
        th = np.asarray(theta).reshape(NFACE, HW).astype(np.float32)
        z = np.asarray(height).reshape(NFACE, HW).astype(np.float32)
    return th, z


def _build_program(escale, ascale, bscale, tstar):
    """One SPMD program; per-core data differs only in the input tensors."""
    nc = bacc.Bacc("TRN2", target_bir_lowering=False, debug=False,
                   num_devices=NCORES)
    f32 = mybir.dt.float32
    bf16 = mybir.dt.float16
    xt_d = nc.dram_tensor("xt", [SEG_PER_CORE, 2, 128, XTW], bf16, kind="ExternalInput")
    wvt_d = nc.dram_tensor("wvt", [2, 128, C], bf16, kind="ExternalInput")
    thw_d = nc.dram_tensor("thw", [SEG_PER_CORE, 1, SEGW], f32, kind="ExternalInput")
    zw_d = nc.dram_tensor("zw", [SEG_PER_CORE, 1, SEGW], f32, kind="ExternalInput")
    thrz_d = nc.dram_tensor("thrz", [SEG_PER_CORE, 128, 2 * NCH], f32, kind="ExternalInput")
    out_d = nc.dram_tensor("out", [SEG_PER_CORE, SEGJ, 128, C], f32, kind="ExternalOutput")

    AF = mybir.ActivationFunctionType
    OP = mybir.AluOpType

    def bcast(ap):  # [1, N] dram AP -> partition-broadcast to 128
        return bass.AP(tensor=ap.tensor, offset=ap.offset,
                       ap=[[0, 128]] + ap.ap[1:])

    with tile.TileContext(nc) as tc:
        with tc.tile_pool(name="const", bufs=1) as constp, \
             tc.tile_pool(name="seg", bufs=3) as segp, \
             tc.tile_pool(name="vsb", bufs=12) as vsbp, \
             tc.tile_pool(name="big", bufs=3) as bigp, \
             tc.tile_pool(name="osb", bufs=4) as osbp, \
             tc.tile_pool(name="vps", bufs=4, space="PSUM") as vpsp, \
             tc.tile_pool(name="ops", bufs=3, space="PSUM") as opsp:

            wv0 = constp.tile([128, C], bf16, tag="wv0")
            wv1 = constp.tile([128, C], bf16, tag="wv1")
            nc.sync.dma_start(out=wv0, in_=wvt_d.ap()[0])
            nc.sync.dma_start(out=wv1, in_=wvt_d.ap()[1])

            # ---- prefetch all segments' inputs on parallel DMA queues ----
            seg_tiles = []
            for si in range(SEG_PER_CORE):
                xt0 = segp.tile([128, XTW], bf16, tag="xt0")
                xt1 = segp.tile([128, XTW], bf16, tag="xt1")
                nc.sync.dma_start(out=xt0, in_=xt_d.ap()[si, 0])
                nc.sync.dma_start(out=xt1, in_=xt_d.ap()[si, 1])
                thwB = segp.tile([128, SEGW], f32, tag="thwB")
                zwB = segp.tile([128, SEGW], f32, tag="zwB")
                # quartered broadcasts: early window chunks become ready as
                # soon as their columns land instead of after the full 459KB
                QW = SEGW // 4
                for q in range(4):
                    qs = slice(q * QW, (q + 1) * QW)
                    nc.scalar.dma_start(out=thwB[:, qs],
                                        in_=bcast(thw_d.ap()[si, :, qs]))
                    nc.scalar.dma_start(out=zwB[:, qs],
                                        in_=bcast(zw_d.ap()[si, :, qs]))
                thrzT = segp.tile([128, 2 * NCH], f32, tag="thrzT")
                nc.gpsimd.dma_start(out=thrzT, in_=thrz_d.ap()[si])
                seg_tiles.append((xt0, xt1, thwB, zwB, thrzT))

            for si in range(SEG_PER_CORE):
                xt0, xt1, thwB, zwB, thrzT = seg_tiles[si]
                thrT = thrzT[:, 0:NCH]
                zrT = thrzT[:, NCH:2 * NCH]

                # ---- v chunks (shifted grid): v[k] = x_chunk @ Wv.T ----
                vsb = []
                for pair in range(NCH // 2):  # 4 pairs
                    vp = vpsp.tile([128, 512], f32, tag="vps")
                    for kk in range(2):
                        k = 2 * pair + kk
                        sl = slice(256 * kk, 256 * kk + 256)
                        nc.tensor.matmul(vp[:, sl], xt0[:, 128 * k:128 * k + 128],
                                         wv0[:], start=True, stop=False)
                        nc.tensor.matmul(vp[:, sl], xt1[:, 128 * k:128 * k + 128],
                                         wv1[:], start=False, stop=True)
                    vt = vsbp.tile([128, 512], bf16, tag="vsb")
                    nc.vector.tensor_copy(vt[:], vp[:])
                    vsb.append(vt)

                def vchunk(k):  # rhs [128, 256] for shifted chunk k (0..7)
                    return vsb[k // 2][:, 256 * (k % 2):256 * (k % 2) + 256]

                # ---- similarity windows, fused across the segment ----
                # partitions = shifted chunk rows (contraction index c),
                # free = [chunk p, window col m]; window p covers ext cols
                # [128(p-1), 128(p+1)) = thwB[:, 128p : 128p+256].
                # Only big-cols [128, 1920) feed the matmuls: W[0]'s left half
                # and W[7]'s right half are never used. TRIMW tiles represent
                # big-col range [128, 1920).
                TRIMW = BIGW - 256  # 1792
                Abig = bigp.tile([128, TRIMW], f32, tag="Abig")
                Bbig = bigp.tile([128, TRIMW], f32, tag="Bbig")
                for jp in range(NCH):
                    w0, w1 = 128 * jp - 128, 128 * jp + WINW - 128
                    o0, o1 = WINW * jp - 128, WINW * jp + WINW - 128
                    if jp == 0:
                        w0 += 128
                        o0 += 128
                    if jp == NCH - 1:
                        w1 -= 128
                        o1 -= 128
                    nc.scalar.activation(out=Abig[:, o0:o1], in_=thwB[:, w0:w1],
                                         func=AF.Square,
                                         bias=thrT[:, jp:jp + 1], scale=ascale)
                    nc.scalar.activation(out=Bbig[:, o0:o1], in_=zwB[:, w0:w1],
                                         func=AF.Square,
                                         bias=zrT[:, jp:jp + 1], scale=bscale)
                D = bigp.tile([128, TRIMW], f32, tag="D")
                E = bigp.tile([128, TRIMW], bf16, tag="E")
                M = bigp.tile([128, TRIMW], bf16, tag="M")
                S = bigp.tile([128, TRIMW], bf16, tag="S")
                # half-segment granularity: pipelines E/M/S (and the sim
                # matmuls) against the later A/B squares
                for h0, h1 in ((0, TRIMW // 2), (TRIMW // 2, TRIMW)):
                    hs = slice(h0, h1)
                    nc.vector.tensor_tensor(out=D[:, hs], in0=Abig[:, hs],
                                            in1=Bbig[:, hs], op=OP.add)
                    nc.scalar.activation(out=E[:, hs], in_=D[:, hs], func=AF.Exp,
                                         scale=escale)
                    nc.vector.tensor_scalar(out=M[:, hs], in0=D[:, hs],
                                            scalar1=tstar, scalar2=None,
                                            op0=OP.is_le)
                    nc.vector.tensor_tensor(out=S[:, hs], in0=E[:, hs],
                                            in1=M[:, hs], op=OP.mult)

                # ---- banded sim @ v:  out[q] = W[q][:,128:256]^T v_q
                #                              + W[q+1][:,0:128]^T v_{q+1} ----
                # S covers big-cols [128, 1920): subtract 128 from slice offsets.
                for j in range(SEGJ):
                    op = opsp.tile([128, C], f32, tag="ops")
                    nc.tensor.matmul(op[:], S[:, WINW * j:WINW * j + 128],
                                     vchunk(j), start=True, stop=False)
                    nc.tensor.matmul(op[:], S[:, WINW * (j + 1) - 128:WINW * (j + 1)],
                                     vchunk(j + 1), start=False, stop=True)
                    ot = osbp.tile([128, C], f32, tag="osb")
                    if si == SEG_PER_CORE - 1 and j % 2 == 1:
                        nc.scalar.copy(ot[:], op[:])
                    else:
                        nc.vector.tensor_copy(ot[:], op[:])
                    nc.sync.dma_start(out=out_d.ap()[si, j], in_=ot)

    nc.compile()
    return nc


def kernel(depth, K, T, cov, x, Wv):
    global LAST_RESULT
    depth = np.asarray(depth)
    K = np.asarray(K)
    T = np.asarray(T)
    cov = np.asarray(cov)
    x = np.asarray(x, dtype=np.float32)
    Wv = np.asarray(Wv, dtype=np.float32)

    th, z = _geometry(depth, K, T)

    icov = np.linalg.inv(cov.astype(np.float64))
    i00, i11 = icov[0, 0], icov[1, 1]
    assert abs(icov[0, 1]) <= 1e-9 * abs(i00) and abs(icov[1, 0]) <= 1e-9 * abs(i00), \
        "kernel assumes diagonal inv_cov"
    escale = float(-0.5 * i00)
    ascale = float(RADIUS)
    bscale = float(np.sqrt(i11 / i00))
    tstar = float(np.float32((TRUNC * TRUNC) / i00))

    # ---- sort each face by theta; build packed per-core inputs ----
    TWO_PI = np.float32(2 * np.pi)
    perms, th_s, z_s, x_srt = [], [], [], []
    thr_half = float(np.sqrt((TRUNC * TRUNC) / i00)) / RADIUS  # |dtheta| bound
    for f in range(NFACE):
        p = np.argsort(th[f], kind="stable")
        ts_, zs_ = th[f][p], z[f][p]
        perms.append(p)
        th_s.append(ts_)
        z_s.append(zs_)
        x_srt.append(x[f][p])  # [HW, C]
        ext = np.concatenate([ts_[-200:] - TWO_PI, ts_, ts_[:200] + TWO_PI])
        pos = np.arange(HW) + 200
        lo = np.searchsorted(ext, ts_ - thr_half)
        hi = np.searchsorted(ext, ts_ + thr_half)
        wmax = max((pos - lo).max(), (hi - 1 - pos).max())
        assert wmax <= 64, f"face {f}: theta band halfwidth {wmax} > 64"

    def ext_th(f, c):  # positions with +-2pi wrap shifts
        c = np.asarray(c)
        out = th_s[f][c % HW].astype(np.float32)
        out = np.where(c < 0, out - TWO_PI, out)
        out = np.where(c >= HW, out + TWO_PI, out)
        return out

    in_maps = []
    wvt = np.ascontiguousarray(Wv.T.reshape(2, 128, C)).astype(BF16)
    for core in range(NCORES):
        xt = np.zeros((SEG_PER_CORE, 2, 128, XTW), BF16)
        thw = np.zeros((SEG_PER_CORE, 1, SEGW), np.float32)
        zw = np.zeros((SEG_PER_CORE, 1, SEGW), np.float32)
        thrz = np.zeros((SEG_PER_CORE, 128, 2 * NCH), np.float32)
        for si in range(SEG_PER_CORE):
            g = core * SEG_PER_CORE + si
            f, s0 = g // 2, 0 if g % 2 == 0 else 6
            wcols = np.arange(128 * s0, 128 * s0 + SEGW)
            thw[si, 0] = ext_th(f, wcols)
            zw[si, 0] = z_s[f][wcols % HW]
            xcols = np.arange(128 * s0 - 64, 128 * s0 - 64 + XTW)
            xt[si] = x_srt[f][xcols % HW].T.reshape(2, 128, XTW).astype(BF16)
            for jp in range(NCH):
                rows = np.arange(128 * (s0 + jp) - 64, 128 * (s0 + jp) + 64)
                thrz[si, :, jp] = -np.float32(ascale) * ext_th(f, rows)
                thrz[si, :, NCH + jp] = -np.float32(bscale) * z_s[f][rows % HW]
        in_maps.append({"xt": xt, "wvt": wvt, "thw": thw, "zw": zw,
                        "thrz": thrz})

    nc = _build_program(escale, ascale, bscale, tstar)
    res = run_bass_kernel_spmd(nc, in_maps, core_ids=list(range(NCORES)))
    LAST_RESULT = res
    if res.exec_time_ns is not None:
        print(f"HW exec time: {res.exec_time_ns} ns")

    # ---- gather: per-core outputs -> faces -> unsort ----
    out = np.empty((NFACE, HW, C), np.float32)
    for f in range(NFACE):
        sorted_out = np.empty((HW, C), np.float32)
        for b in range(NBLK):
            if b <= 6:
                g, j = 2 * f, b
            else:
                g, j = 2 * f + 1, b - 6
            core, si = g // SEG_PER_CORE, g % SEG_PER_CORE
            r0 = 128 * b
            n = min(128, HW - r0)
            sorted_out[r0:r0 + n] = res.results[core]["out"][si, j, :n]
        out[f][perms[f]] = sorted_out
    return out


# revision 30
# speedup vs baseline: 1.1593x; 1.1593x over previous
"""Trainium2 Bass kernel for nn_CVT_12446815223882 (sparse cylindrical attention).

Pipeline: unproject depth -> cylinder coords (theta, z) per camera face,
truncated-Gaussian similarity over (theta, z) with inv_cov = (1/0.002) I,
out = sim @ (x @ Wv.T).

Key structure exploited: with cov = 0.002 I and TRUNC = 1.2, the mask
dist2 <= TRUNC^2 keeps only pairs with |dtheta|, |dz| <= 0.0537 -- the
similarity matrix is ~99.9% sparse. Host sorts each face's 1600 points by
theta; surviving pairs then live in a +-36-position band. The device
computes a 256-wide circular band of sim per 128-point chunk and does
banded matmuls. Wrap pairs (theta ~ +-pi) are handled by extending the
sorted arrays with +-2pi-shifted copies -- no mod/wrap ops on device.

Banded layout: out-blocks q live on the 128 grid (rows [128q, 128q+128));
similarity tiles live on a 64-shifted grid (chunks c_p = [128p-64,
128p+64)). Window tile W[p] = sim(c_p rows as partitions, cols [128(p-1),
128(p+1)) as free) covers all |row-col| <= 64 pairs; out[q] contracts
over shifted chunks {q, q+1}, whose lhsT tiles are plain slices of W[q]
and W[q+1] by symmetry of sim.

Work split: each face = 13 out-blocks; 2 overlapping segments of 7 blocks
-> 24 segments = 8 NeuronCores x 3 segments. One SPMD program.
"""

import numpy as np
import ml_dtypes

import concourse.bass as bass
import concourse.bacc as bacc
import concourse.tile as tile
import concourse.mybir as mybir
from concourse.bass_utils import run_bass_kernel_spmd

# ---- problem constants (from the nn.Module definition) ----
R_FIXED = 2.0
TRUNC = 1.2
RADIUS = 1.0
DEPTH_THRESHOLD = 500.0
CENTER = np.array([0.0, 0.0, 1.5], dtype=np.float32)

B_, NC_, H_, W_ = 2, 6, 40, 40
HW = H_ * W_          # 1600
C = 256
NFACE = B_ * NC_      # 12
NBLK = 13             # out row blocks of 128 per face (last 64 real)
SEGJ = 7              # out blocks per segment
NCH = SEGJ + 1        # shifted sim/v chunks per segment
NSEG = NFACE * 2      # 24 (blocks 0..6 and 6..12; block 6 overlaps)
NCORES = 8
SEG_PER_CORE = NSEG // NCORES  # 3
WINW = 256            # window width per shifted chunk
SEGW = 128 * SEGJ         # 896: col span actually read by A/B windows
XTW = 128 * NCH           # 1024: x cols per segment (shifted grid)
BIGW = WINW * NCH         # 2048: fused elementwise width

LAST_RESULT = None
BF16 = np.float16


def _geometry(depth, K, T):
    """Replicate reference.py unprojection + cylinder projection bit-for-bit
    (jax on CPU, same op sequence) -> theta, z per face [NFACE, HW] f32."""
    import jax
    import jax.numpy as jnp

    cpu = jax.devices("cpu")[0]
    with jax.default_device(cpu):
        depth = jnp.asarray(np.asarray(depth), jnp.float32)
        K = jnp.asarray(np.asarray(K), jnp.float32)
        T = jnp.asarray(np.asarray(T), jnp.float32)
        Bb, NC, _, H, W = depth.shape
        ys, xs = jnp.meshgrid(jnp.arange(H, dtype=jnp.float32),
                              jnp.arange(W, dtype=jnp.float32), indexing="ij")
        d = depth[:, :, 0]
        fx = K[:, :, 0, 0][..., None, None]
        fy = K[:, :, 1, 1][..., None, None]
        cx = K[:, :, 0, 2][..., None, None]
        cy = K[:, :, 1, 2][..., None, None]
        X = (xs - cx) / fx * d
        Y = (ys - cy) / fy * d
        pts_cam = jnp.stack([X, Y, d], axis=-1).reshape(Bb, NC, H * W, 3)
        valid = ((d > 0) & (d <= DEPTH_THRESHOLD)).reshape(Bb, NC, H * W)
        pts_h = jnp.concatenate([pts_cam, jnp.ones_like(pts_cam[..., :1])], axis=-1)
        pts_ego = jnp.einsum("bnkj,bnij->bnki", pts_h, T)[..., :3]
        pts_ego = jnp.where(valid[..., None], pts_ego, 0.0)
        center = jnp.asarray(CENTER)
        v = pts_ego - center
        r = jnp.sqrt(v[..., 0] ** 2 + v[..., 1] ** 2)
        r_safe = jnp.where(r == 0, 1e-6, r)
        t = (R_FIXED / r_safe)[..., None]
        proj = center - t * v
        theta = jnp.arctan2(proj[..., 1] - center[1], proj[..., 0] - center[0])
        height = proj[..., 2] - center[2]
        th = np.asarray(theta).reshape(NFACE, HW).astype(np.float32)
        z = np.asarray(height).reshape(NFACE, HW).astype(np.float32)
    return th, z


def _build_program(escale, ascale, bscale, tstar):
    """One SPMD program; per-core data differs only in the input tensors."""
    nc = bacc.Bacc("TRN2", target_bir_lowering=False, debug=False,
                   num_devices=NCORES)
    f32 = mybir.dt.float32
    bf16 = mybir.dt.float16
    xt_d = nc.dram_tensor("xt", [SEG_PER_CORE, 2, 128, XTW], bf16, kind="ExternalInput")
    wvt_d = nc.dram_tensor("wvt", [2, 128, C], bf16, kind="ExternalInput")
    thw_d = nc.dram_tensor("thw", [SEG_PER_CORE, 1, SEGW], f32, kind="ExternalInput")
    zw_d = nc.dram_tensor("zw", [SEG_PER_CORE, 1, SEGW], f32, kind="ExternalInput")
    thrz_d = nc.dram_tensor("thrz", [SEG_PER_CORE, 128, 2 * NCH], f32, kind="ExternalInput")
    out_d = nc.dram_tensor("out", [SEG_PER_CORE, SEGJ, 128, C], f32, kind="ExternalOutput")

    AF = mybir.ActivationFunctionType
    OP = mybir.AluOpType

    def bcast(ap):  # [1, N] dram AP -> partition-broadcast to 128
        return bass.AP(tensor=ap.tensor, offset=ap.offset,
                       ap=[[0, 128]] + ap.ap[1:])

    with tile.TileContext(nc) as tc:
        with tc.tile_pool(name="const", bufs=1) as constp, \
             tc.tile_pool(name="seg", bufs=3) as segp, \
             tc.tile_pool(name="vsb", bufs=12) as vsbp, \
             tc.tile_pool(name="big", bufs=3) as bigp, \
             tc.tile_pool(name="osb", bufs=4) as osbp, \
             tc.tile_pool(name="vps", bufs=4, space="PSUM") as vpsp, \
             tc.tile_pool(name="ops", bufs=3, space="PSUM") as opsp:

            wv0 = constp.tile([128, C], bf16, tag="wv0")
            wv1 = constp.tile([128, C], bf16, tag="wv1")
            nc.sync.dma_start(out=wv0, in_=wvt_d.ap()[0])
            nc.sync.dma_start(out=wv1, in_=wvt_d.ap()[1])

            # ---- prefetch all segments' inputs on parallel DMA queues ----
            seg_tiles = []
            for si in range(SEG_PER_CORE):
                xt0 = segp.tile([128, XTW], bf16, tag="xt0")
                xt1 = segp.tile([128, XTW], bf16, tag="xt1")
                nc.sync.dma_start(out=xt0, in_=xt_d.ap()[si, 0])
                nc.sync.dma_start(out=xt1, in_=xt_d.ap()[si, 1])
                thwB = segp.tile([128, SEGW], f32, tag="thwB")
                zwB = segp.tile([128, SEGW], f32, tag="zwB")
                nc.scalar.dma_start(out=thwB, in_=bcast(thw_d.ap()[si]))
                nc.sync.dma_start(out=zwB, in_=bcast(zw_d.ap()[si]))
                thrzT = segp.tile([128, 2 * NCH], f32, tag="thrzT")
                nc.gpsimd.dma_start(out=thrzT, in_=thrz_d.ap()[si])
                seg_tiles.append((xt0, xt1, thwB, zwB, thrzT))

            for si in range(SEG_PER_CORE):
                xt0, xt1, thwB, zwB, thrzT = seg_tiles[si]
                thrT = thrzT[:, 0:NCH]
                zrT = thrzT[:, NCH:2 * NCH]

                # ---- v chunks (shifted grid): v[k] = x_chunk @ Wv.T ----
                vsb = []
                for pair in range(NCH // 2):  # 4 pairs
                    vp = vpsp.tile([128, 512], f32, tag="vps")
                    for kk in range(2):
                        k = 2 * pair + kk
                        sl = slice(256 * kk, 256 * kk + 256)
                        nc.tensor.matmul(vp[:, sl], xt0[:, 128 * k:128 * k + 128],
                                         wv0[:], start=True, stop=False)
                        nc.tensor.matmul(vp[:, sl], xt1[:, 128 * k:128 * k + 128],
                                         wv1[:], start=False, stop=True)
                    vt = vsbp.tile([128, 512], bf16, tag="vsb")
                    nc.vector.tensor_copy(vt[:], vp[:])
                    vsb.append(vt)

                def vchunk(k):  # rhs [128, 256] for shifted chunk k (0..7)
                    return vsb[k // 2][:, 256 * (k % 2):256 * (k % 2) + 256]

                # ---- similarity windows, fused across the segment ----
                # partitions = shifted chunk rows (contraction index c),
                # free = [chunk p, window col m]; window p covers ext cols
                # [128(p-1), 128(p+1)) = thwB[:, 128p : 128p+256].
                # Only big-cols [128, 1920) feed the matmuls: W[0]'s left half
                # and W[7]'s right half are never used. TRIMW tiles represent
                # big-col range [128, 1920).
                TRIMW = BIGW - 256  # 1792
                Abig = bigp.tile([128, TRIMW], f32, tag="Abig")
                Bbig = bigp.tile([128, TRIMW], f32, tag="Bbig")
                for jp in range(NCH):
                    w0, w1 = 128 * jp - 128, 128 * jp + WINW - 128
                    o0, o1 = WINW * jp - 128, WINW * jp + WINW - 128
                    if jp == 0:
                        w0 += 128
                        o0 += 128
                    if jp == NCH - 1:
                        w1 -= 128
                        o1 -= 128
                    nc.scalar.activation(out=Abig[:, o0:o1], in_=thwB[:, w0:w1],
                                         func=AF.Square,
                                         bias=thrT[:, jp:jp + 1], scale=ascale)
                    nc.scalar.activation(out=Bbig[:, o0:o1], in_=zwB[:, w0:w1],
                                         func=AF.Square,
                                         bias=zrT[:, jp:jp + 1], scale=bscale)
                D = bigp.tile([128, TRIMW], f32, tag="D")
                E = bigp.tile([128, TRIMW], bf16, tag="E")
                M = bigp.tile([128, TRIMW], bf16, tag="M")
                S = bigp.tile([128, TRIMW], bf16, tag="S")
                # half-segment granularity: pipelines E/M/S (and the sim
                # matmuls) against the later A/B squares
                for h0, h1 in ((0, TRIMW // 2), (TRIMW // 2, TRIMW)):
                    hs = slice(h0, h1)
                    nc.vector.tensor_tensor(out=D[:, hs], in0=Abig[:, hs],
                                            in1=Bbig[:, hs], op=OP.add)
                    nc.scalar.activation(out=E[:, hs], in_=D[:, hs], func=AF.Exp,
                                         scale=escale)
                    nc.vector.tensor_scalar(out=M[:, hs], in0=D[:, hs],
                                            scalar1=tstar, scalar2=None,
                                            op0=OP.is_le)
                    nc.vector.tensor_tensor(out=S[:, hs], in0=E[:, hs],
                                            in1=M[:, hs], op=OP.mult)

                # ---- banded sim @ v:  out[q] = W[q][:,128:256]^T v_q
                #                              + W[q+1][:,0:128]^T v_{q+1} ----
                # S covers big-cols [128, 1920): subtract 128 from slice offsets.
                for j in range(SEGJ):
                    op = opsp.tile([128, C], f32, tag="ops")
                    nc.tensor.matmul(op[:], S[:, WINW * j:WINW * j + 128],
                                     vchunk(j), start=True, stop=False)
                    nc.tensor.matmul(op[:], S[:, WINW * (j + 1) - 128:WINW * (j + 1)],
                                     vchunk(j + 1), start=False, stop=True)
                    ot = osbp.tile([128, C], f32, tag="osb")
                    if si == SEG_PER_CORE - 1 and j % 2 == 1:
                        nc.scalar.copy(ot[:], op[:])
                    else:
                        nc.vector.tensor_copy(ot[:], op[:])
                    nc.sync.dma_start(out=out_d.ap()[si, j], in_=ot)

    nc.compile()
    return nc


def kernel(depth, K, T, cov, x, Wv):
    global LAST_RESULT
    depth = np.asarray(depth)
    K = np.asarray(K)
    T = np.asarray(T)
    cov = np.asarray(cov)
    x = np.asarray(x, dtype=np.float32)
    Wv = np.asarray(Wv, dtype=np.float32)

    th, z = _geometry(depth, K, T)

    icov = np.linalg.inv(cov.astype(np.float64))
    i00, i11 = icov[0, 0], icov[1, 1]
    assert abs(icov[0, 1]) <= 1e-9 * abs(i00) and abs(icov[1, 0]) <= 1e-9 * abs(i00), \
        "kernel assumes diagonal inv_cov"
    escale = float(-0.5 * i00)
    ascale = float(RADIUS)
    bscale = float(np.sqrt(i11 / i00))
    tstar = float(np.float32((TRUNC * TRUNC) / i00))

    # ---- sort each face by theta; build packed per-core inputs ----
    TWO_PI = np.float32(2 * np.pi)
    perms, th_s, z_s, x_srt = [], [], [], []
    thr_half = float(np.sqrt((TRUNC * TRUNC) / i00)) / RADIUS  # |dtheta| bound
    for f in range(NFACE):
        p = np.argsort(th[f], kind="stable")
        ts_, zs_ = th[f][p], z[f][p]
        perms.append(p)
        th_s.append(ts_)
        z_s.append(zs_)
        x_srt.append(x[f][p])  # [HW, C]
        ext = np.concatenate([ts_[-200:] - TWO_PI, ts_, ts_[:200] + TWO_PI])
        pos = np.arange(HW) + 200
        lo = np.searchsorted(ext, ts_ - thr_half)
        hi = np.searchsorted(ext, ts_ + thr_half)
        wmax = max((pos - lo).max(), (hi - 1 - pos).max())
        assert wmax <= 64, f"face {f}: theta band halfwidth {wmax} > 64"

    def ext_th(f, c):  # positions with +-2pi wrap shifts
        c = np.asarray(c)
        out = th_s[f][c % HW].astype(np.float32)
        out = np.where(c < 0, out - TWO_PI, out)
        out = np.where(c >= HW, out + TWO_PI, out)
        return out

    in_maps = []
    wvt = np.ascontiguousarray(Wv.T.reshape(2, 128, C)).astype(BF16)
    for core in range(NCORES):
        xt = np.zeros((SEG_PER_CORE, 2, 128, XTW), BF16)
        thw = np.zeros((SEG_PER_CORE, 1, SEGW), np.float32)
        zw = np.zeros((SEG_PER_CORE, 1, SEGW), np.float32)
        thrz = np.zeros((SEG_PER_CORE, 128, 2 * NCH), np.float32)
        for si in range(SEG_PER_CORE):
            g = core * SEG_PER_CORE + si
            f, s0 = g // 2, 0 if g % 2 == 0 else 6
            wcols = np.arange(128 * s0, 128 * s0 + SEGW)
            thw[si, 0] = ext_th(f, wcols)
            zw[si, 0] = z_s[f][wcols % HW]
            xcols = np.arange(128 * s0 - 64, 128 * s0 - 64 + XTW)
            xt[si] = x_srt[f][xcols % HW].T.reshape(2, 128, XTW).astype(BF16)
            for jp in range(NCH):
                rows = np.arange(128 * (s0 + jp) - 64, 128 * (s0 + jp) + 64)
                thrz[si, :, jp] = -np.float32(ascale) * ext_th(f, rows)
                thrz[si, :, NCH + jp] = -np.float32(bscale) * z_s[f][rows % HW]
        in_maps.append({"xt": xt, "wvt": wvt, "thw": thw, "zw": zw,
                        "thrz": thrz})

    nc = _build_program(escale, ascale, bscale, tstar)
    res = run_bass_kernel_spmd(nc, in_maps, core_ids=list(range(NCORES)))
    LAST_RESULT = res
    if res.exec_time_ns is not None:
        print(f"HW exec time: {res.exec_time_ns} ns")

    # ---- gather: per-core outputs -> faces -> unsort ----
    out = np.empty((NFACE, HW, C), np.float32)
    for f in range(NFACE):
        sorted_out = np.empty((HW, C), np.float32)
        for b in range(NBLK):
            if b <= 6:
                g, j = 2 * f, b
            else:
                g, j = 2 * f + 1, b - 6
            core, si = g // SEG_PER_CORE, g % SEG_PER_CORE
            r0 = 128 * b
            n = min(128, HW - r0)
            sorted_out[r0:r0 + n] = res.results[core]["out"][si, j, :n]
        out[f][perms[f]] = sorted_out
    return out
